# revision 3
# baseline (speedup 1.0000x reference)
"""Trainium2 Bass kernel for nn_LocalGlobalRegistration (topk_masking).

Reference computation (per full input score_mat (4096, 64, 64) f32):
  - ref_score_mat: keep per-row (over s) top-3 values in place, else 0
  - src_score_mat: keep per-col (over r) top-3 values in place, else 0
  - global top-2000 of flattened score -> corr_mat (bool scatter) and
    sel_score_mat (value scatter)
  - out_float = ref_score_mat + src_score_mat + sel_score_mat   (masks all 1s)
Returns (corr_mat bool (B,R,S), out_float f32 (B,R,S)).

Device strategy (data-parallel over batch, 512 batches/core on 8 cores):
  Per (128,128) tile = 4 batches, partition=(b&1)*64+r, free=((b>>1)&1)*64+s:
    - Max8 per 64-slice -> top-8 per row (exact, with multiplicity)
    - STT: refk = (x >= m3_row) * x          (m3 = 3rd largest, rank-2 slot)
    - PE transpose -> per-column layout; Max8 + STT again for columns
    - PE transpose back + accumulate refk via identity matmul in PSUM
    - out_tile = refk + srck
  Top-8 value tables (ref8/src8) are DMA'd out; the host merges the global
  top-2000 from them (indices recovered by rescanning candidate rows of the
  host-resident input) and patches the rare rows/cols where the 3rd and 4th
  largest are exactly equal (float tie at the top-k boundary), reproducing
  jax.lax.top_k's lowest-index tie-breaking bit-exactly.
"""

import os
import sys

import numpy as np

sys.path.insert(0, "/opt/trn_rl_repo")

N_CORES = 8
B, R, S = 4096, 64, 64
BPC = B // N_CORES  # batches per core

K_TOPK = 3
NUM_CORR = 2000


# ---------------------------------------------------------------------------
# Device kernel construction
# ---------------------------------------------------------------------------

def build_nc(bpc=BPC):
    """Build the per-core Bass program (SPMD: same program, different data)."""
    from concourse import bacc, mybir
    from concourse import tile
    from concourse import masks

    f32 = mybir.dt.float32
    nt = bpc // 4  # (128,128) tiles, 4 batches each

    nc = bacc.Bacc("TRN2", target_bir_lowering=False, debug=True)

    score_d = nc.dram_tensor("score", [bpc, R, S], f32, kind="ExternalInput")
    out_d = nc.dram_tensor("out", [bpc, R, S], f32, kind="ExternalOutput")
    m8r_d = nc.dram_tensor("m8ref", [128, nt * 16], f32, kind="ExternalOutput")
    m8s_d = nc.dram_tensor("m8src", [128, nt * 16], f32, kind="ExternalOutput")

    ge = mybir.AluOpType.is_ge
    mult = mybir.AluOpType.mult

    with tile.TileContext(nc) as tc:
        with (
            tc.tile_pool(name="const", bufs=1) as constp,
            tc.tile_pool(name="xin", bufs=4) as xpool,
            tc.tile_pool(name="refk", bufs=4) as rpool,
            tc.tile_pool(name="xt", bufs=4) as tpool,
            tc.tile_pool(name="srck", bufs=4) as spool,
            tc.tile_pool(name="outsb", bufs=4) as opool,
            tc.tile_pool(name="pt", bufs=3, space="PSUM") as ptpool,
            tc.tile_pool(name="p2", bufs=3, space="PSUM") as p2pool,
        ):
            ident = constp.tile([128, 128], f32)
            masks.make_identity(nc, ident[:])
            m8r_buf = constp.tile([128, nt * 16], f32)
            m8s_buf = constp.tile([128, nt * 16], f32)

            for j in range(nt):
                hbm_in = score_d[4 * j : 4 * j + 4].rearrange(
                    "(j2 b2) r s -> (b2 r) j2 s", j2=2, b2=2
                )
                x2 = xpool.tile([128, 128], f32)
                nc.sync.dma_start(
                    out=x2[:].rearrange("p (j2 s) -> p j2 s", j2=2), in_=hbm_in
                )

                refk = rpool.tile([128, 128], f32)
                for j2 in (0, 1):
                    sl = slice(j2 * 64, j2 * 64 + 64)
                    k8 = (2 * j + j2) * 8
                    nc.vector.max(m8r_buf[:, k8 : k8 + 8], x2[:, sl])
                    nc.vector.scalar_tensor_tensor(
                        out=refk[:, sl],
                        in0=x2[:, sl],
                        scalar=m8r_buf[:, k8 + 2 : k8 + 3],
                        in1=x2[:, sl],
                        op0=ge,
                        op1=mult,
                    )

                # transpose x2 -> xt (partition=(j2,s), free=(b2,r))
                pt = ptpool.tile([128, 128], f32)
                nc.tensor.matmul(pt[:], x2[:], ident[:], is_transpose=True)
                xt = tpool.tile([128, 128], f32)
                nc.scalar.copy(out=xt[:], in_=pt[:])

                srck = spool.tile([128, 128], f32)
                for b2 in (0, 1):
                    sl = slice(b2 * 64, b2 * 64 + 64)
                    k8 = (2 * j + b2) * 8
                    nc.vector.max(m8s_buf[:, k8 : k8 + 8], xt[:, sl])
                    nc.vector.scalar_tensor_tensor(
                        out=srck[:, sl],
                        in0=xt[:, sl],
                        scalar=m8s_buf[:, k8 + 2 : k8 + 3],
                        in1=xt[:, sl],
                        op0=ge,
                        op1=mult,
                    )

                # transpose srck back and accumulate refk: p2 = srck.T + I.T@refk
                p2 = p2pool.tile([128, 128], f32)
                nc.tensor.matmul(
                    p2[:], srck[:], ident[:], is_transpose=True,
                    start=True, stop=False, skip_group_check=True,
                )
                nc.tensor.matmul(
                    p2[:], ident[:], refk[:],
                    start=False, stop=True, skip_group_check=True,
                )
                outsb = opool.tile([128, 128], f32)
                nc.scalar.copy(out=outsb[:], in_=p2[:])

                hbm_out = out_d[4 * j : 4 * j + 4].rearrange(
                    "(j2 b2) r s -> (b2 r) j2 s", j2=2, b2=2
                )
                nc.sync.dma_start(
                    out=hbm_out, in_=outsb[:].rearrange("p (j2 s) -> p j2 s", j2=2)
                )

            nc.sync.dma_start(out=m8r_d[:], in_=m8r_buf[:])
            nc.sync.dma_start(out=m8s_d[:], in_=m8s_buf[:])

    nc.compile()
    return nc


_NC_CACHE = {}


def _get_nc(bpc=BPC):
    if bpc not in _NC_CACHE:
        _NC_CACHE[bpc] = build_nc(bpc)
    return _NC_CACHE[bpc]


def _decode_m8ref(arr, nt):
    # arr (128, nt*16): [b2*64+r, (2j+j2)*8+q] -> (4j+2*j2+b2, r, q)
    a = arr.reshape(2, 64, nt, 2, 8)  # [b2, r, j, j2, q]
    return np.ascontiguousarray(a.transpose(2, 3, 0, 1, 4).reshape(nt * 4, 64, 8))


def _decode_m8src(arr, nt):
    # arr (128, nt*16): [j2*64+s, (2j+b2)*8+q] -> (4j+2*j2+b2, s, q)
    a = arr.reshape(2, 64, nt, 2, 8)  # [j2, s, j, b2, q]
    return np.ascontiguousarray(a.transpose(2, 0, 3, 1, 4).reshape(nt * 4, 64, 8))


def run_device(score, bpc=BPC, trace=False):
    """Run the bass kernel on the 8 NeuronCores over the full score array.

    Returns (out_partial (B,R,S) f32, ref8 (B,R,8), src8 (B,S,8), exec_time_ns)
    """
    from concourse.bass_utils import run_bass_kernel_spmd

    nb = score.shape[0]
    assert nb % N_CORES == 0 and nb // N_CORES == bpc
    nt = bpc // 4
    nc = _get_nc(bpc)
    shards = [
        np.ascontiguousarray(score[c * bpc : (c + 1) * bpc]) for c in range(N_CORES)
    ]
    in_maps = [{"score": sh} for sh in shards]
    res = run_bass_kernel_spmd(nc, in_maps, list(range(N_CORES)), trace=trace)
    outs = np.concatenate([res.results[c]["out"] for c in range(N_CORES)], axis=0)
    ref8 = np.concatenate(
        [_decode_m8ref(res.results[c]["m8ref"], nt) for c in range(N_CORES)], axis=0
    )
    src8 = np.concatenate(
        [_decode_m8src(res.results[c]["m8src"], nt) for c in range(N_CORES)], axis=0
    )
    return outs, ref8, src8, res.exec_time_ns


# ---------------------------------------------------------------------------
# Host-side finalization (exact tie-break fixups + global top-2000 merge)
# ---------------------------------------------------------------------------

def _exact_topk_keep(vec, k=K_TOPK):
    """Keep top-k of 1-D vec in place (lax.top_k lowest-index tie-break)."""
    order = np.argsort(-vec, kind="stable")[:k]
    kept = np.zeros_like(vec)
    kept[order] = vec[order]
    return kept


def _finalize_host(score, out_f, ref8, src8):
    b, r, s = score.shape

    # --- fix rows where the top-3 boundary has an exact value tie ---
    bad = np.argwhere(ref8[:, :, 2] == ref8[:, :, 3])
    for bb, rr in bad:
        row = score[bb, rr, :]
        dev = row * (row >= ref8[bb, rr, 2])
        out_f[bb, rr, :] += _exact_topk_keep(row) - dev
    bad = np.argwhere(src8[:, :, 2] == src8[:, :, 3])
    for bb, ss in bad:
        col = score[bb, :, ss]
        dev = col * (col >= src8[bb, ss, 2])
        out_f[bb, :, ss] += _exact_topk_keep(col) - dev

    # --- global top-NUM_CORR via per-row top-8 tables ---
    flat8 = ref8.reshape(-1)
    t_cand = np.partition(flat8, flat8.size - NUM_CORR)[flat8.size - NUM_CORR]
    cand_rows = np.argwhere(ref8[:, :, 0] >= t_cand)
    vals = []
    idxs = []
    for bb, rr in cand_rows:
        row = score[bb, rr, :]
        hit = np.nonzero(row >= t_cand)[0]
        vals.append(row[hit])
        idxs.append(bb * (r * s) + rr * s + hit)
    vals = np.concatenate(vals)
    idxs = np.concatenate(idxs)
    assert vals.size >= NUM_CORR
    order = np.lexsort((idxs, -vals))[:NUM_CORR]
    sel_idx = idxs[order]
    sel_val = vals[order]

    corr = np.zeros(b * r * s, dtype=bool)
    corr[sel_idx] = True
    out_f.reshape(-1)[sel_idx] += sel_val
    return corr.reshape(b, r, s), out_f


def _numpy_reference(score_mat, ref_knn_masks, src_knn_masks):
    """Pure-numpy fallback replicating reference.py (used only if masks
    are not all ones, which the fixed setup_inputs never produces)."""
    b, r, s = score_mat.shape
    mask = (ref_knn_masks[:, :, None] & src_knn_masks[:, None, :])
    x = score_mat.astype(np.float32)

    def topk_keep(a, axis):
        mv = np.moveaxis(a, axis, -1)
        flat = mv.reshape(-1, mv.shape[-1])
        kept = np.zeros_like(flat)
        order = np.argsort(-flat, axis=1, kind="stable")[:, :K_TOPK]
        rows = np.arange(flat.shape[0])[:, None]
        kept[rows, order] = flat[rows, order]
        return np.moveaxis(kept.reshape(mv.shape), -1, axis)

    refm = topk_keep(x, 2)
    srcm = topk_keep(x, 1)
    flat = x.reshape(-1)
    order = np.lexsort((np.arange(flat.size), -flat))[:NUM_CORR]
    corr = np.zeros(flat.size, dtype=bool)
    corr[order] = True
    sel = np.zeros(flat.size, dtype=np.float32)
    sel[order] = flat[order]
    corr = corr.reshape(b, r, s) & mask
    out = (refm + srcm + sel.reshape(b, r, s)) * mask.astype(np.float32)
    return corr, out


def kernel(score_mat, ref_knn_masks, src_knn_masks):
    score = np.ascontiguousarray(np.asarray(score_mat, dtype=np.float32))
    rm = np.asarray(ref_knn_masks)
    sm = np.asarray(src_knn_masks)
    if not (rm.all() and sm.all()):
        return _numpy_reference(score, rm, sm)

    out_f, ref8, src8, _ = run_device(score)
    corr, out_f = _finalize_host(score, out_f, ref8, src8)
    return corr, out_f


if __name__ == "__main__":
    # quick smoke: tiny sim run
    rng = np.random.default_rng(0)
    score = (rng.integers(0, 1 << 23, (16, R, S)) / float(1 << 23)).astype(np.float32)
    from concourse.bass_interp import CoreSim

    nc = build_nc(16)
    sim = CoreSim(nc)
    sim.tensor("score")[:] = score
    sim.simulate()
    out = np.array(sim.tensor("out"))
    ref8 = _decode_m8ref(np.array(sim.tensor("m8ref")), 4)
    src8 = _decode_m8src(np.array(sim.tensor("m8src")), 4)

    # numpy check of device math
    m3r = np.sort(score, axis=2)[:, :, ::-1][:, :, :8]
    m3s = np.sort(score, axis=1)[:, ::-1, :][:, :8, :].transpose(0, 2, 1)
    np.testing.assert_array_equal(ref8, m3r)
    np.testing.assert_array_equal(src8, m3s)
    exp = score * (score >= m3r[:, :, 2:3]) + score * (
        score >= m3s[:, :, 2].transpose()[None].reshape(16, 1, 64)
    )
    exp = score * (score >= m3r[:, :, 2:3]) + score * (score >= m3s[:, :, 2][:, None, :])
    np.testing.assert_allclose(out, exp, rtol=0, atol=0)
    print("SIM OK")
